# revision 20
# baseline (speedup 1.0000x reference)
"""Multi-head attention forward on 8 TRN2 NeuronCores — tensor-parallel.

Problem: B=4, S=2048, D=1024, H=16, Hd=64, fp32.
  qkv = x @ w_qkv + b_qkv ; per-head softmax(q k^T / 8) v ; out proj.

Sharding: head-parallel. Core c owns heads 2c, 2c+1 and receives the
FULL x (replicated input); it computes q,k,v for its two heads over all
B*S = 8192 rows locally, so attention needs NO collective at all. The
only communication is a small AllToAll of the normalized per-head
attention outputs (bf16, 2 x 1 MB per core) that reshards from
head-blocks to row-blocks, after which the out-projection is fully
local over the core's 1024 output rows (batch c//2, s-half c%2).
There is ONE AllToAll per iteration; its dependent out-projection is
emitted one iteration late so the PE queue never waits on the
collective, and iterations pipeline.

Precision: everything bf16 with fp32 PSUM accumulation. NOTE: random
per-element perturbations of the attention weights do NOT wash out in
the output (signal and error both scale as sqrt(sum a^2) for random v),
so fp8 anywhere in the q/k/score path costs ~3-5% rel err — measured,
not theoretical. bf16 (~0.3%) is the floor that fits the 2e-2 gate.

Layouts (feature-on-partition; h0 on partitions 0:64, h1 on 64:128):
  xT       [1024, 8192]   full x, transposed on host (bf16)
  qT/kT[b] [128, 2048]    per-batch projections; UNSCALED q (the
                          1/sqrt(Hd) rides the exp free affine)
  v65[b]   [128, 16*130]  per k-tile: [v_h0(64)|1|v_h1(64)|1] so PSUM
                          row 64 of each head's PV yields the softmax
                          denominator for free. Written directly from
                          the v-proj PSUM ([128 rows, 128 feat],
                          x-tile stationary) — no PE transposes.
  sc       [128 k, 1024]  fp32 PSUM (2 banks): h0 cols 0:512, h1
                          512:1024; ONE FD=1024 exp per k-tile covers
                          both heads (halves ACT instruction count).
  otp      [65, 1024]     fp32 PSUM (2 banks): PV accumulator for both
                          heads; row 64 = denominator. Evacuated to
                          SBUF by one DVE copy so the banks free fast;
                          normalization runs from SBUF.
  staging  [128, 512]     normalized attn-out, both heads -> A2A shard
                          (shard j = 2b + qc//2, columns (qc%2)*512)

The two heads' QK matmuls are emitted adjacently so they run
concurrently as 64-row PE tiles (h0 partitions 0:64 -> tile T0, h1
64:128 -> T8). PSUM budget: scores 2x2 banks + otp 1x2 + proj/out-proj
mm 2 = 8. Out-projection is all-bf16 (attn-out used straight from the
A2A receive, no fp32 upconvert); its bias is a broadcast DVE add fused
into the PSUM evacuation, not a matmul.
"""

import sys

import numpy as np

for _p in ("/opt/trn_rl_repo",):
    if _p not in sys.path:
        sys.path.insert(0, _p)

B, S, D = 4, 2048, 1024
H, HD = 16, 64
NC = 8
ROWS = B * S           # 8192 total rows
MYROWS = ROWS // NC    # 1024 output rows per core
KT = D // 128          # 8 k-tiles over D

_CACHE = {}


def _build(iters=1, ablate=(), single=False):
    import concourse.bass as bass  # noqa: F401
    import concourse.mybir as mybir
    from concourse import bacc, tile

    dt = mybir.dt
    f32, bf16 = dt.float32, dt.bfloat16
    AF = mybir.ActivationFunctionType

    nc = bacc.Bacc("TRN2", target_bir_lowering=False, debug=False,
                   num_devices=(1 if single else NC))

    xT = nc.dram_tensor("xT", [D, ROWS], bf16, kind="ExternalInput")
    wqkv = nc.dram_tensor("wqkv", [D, 384], bf16, kind="ExternalInput")
    bqkv = nc.dram_tensor("bqkv", [384, 1], f32, kind="ExternalInput")
    bvrow = nc.dram_tensor("bvrow", [1, 512], f32, kind="ExternalInput")
    wout = nc.dram_tensor("wout", [D, D], bf16, kind="ExternalInput")
    bout = nc.dram_tensor("bout", [1, D], f32, kind="ExternalInput")
    onesb = nc.dram_tensor("onesb", [128, 32], bf16, kind="ExternalInput")
    out = nc.dram_tensor("out", [MYROWS, D], f32, kind="ExternalOutput")

    SK = 16                 # key tiles of 128 per batch
    with tile.TileContext(nc) as tc:
        with (
            tc.tile_pool(name="persist", bufs=1) as pp,
            tc.tile_pool(name="dram", bufs=1, space="DRAM") as dp,
        ):
            wq_sb = pp.tile([128, KT * 384], bf16)
            bq_sb = pp.tile([128, 3], f32)
            bv_row = pp.tile([1, 512], f32)
            bv_bc = pp.tile([128, 512], f32)
            wout_sb = pp.tile([128, KT * 1024], bf16)
            bout_sb = pp.tile([1, 1024], f32)
            bout_bc = pp.tile([128, 1024], f32)
            qT = [pp.tile([128, S], bf16, name=f"qT{b}") for b in range(B)]
            kT = [pp.tile([128, S], bf16, name=f"kT{b}") for b in range(B)]
            v65 = [pp.tile([128, SK * 130], bf16, name=f"v65_{b}")
                   for b in range(B)]

            nc.sync.dma_start(
                out=wq_sb[:].rearrange("p (k m) -> p k m", m=384),
                in_=wqkv.ap().rearrange("(k p) m -> p k m", p=128))
            nc.sync.dma_start(
                out=bq_sb[:],
                in_=bqkv.ap().rearrange("(m p) o -> p (m o)", p=128))
            nc.sync.dma_start(out=bv_row[:], in_=bvrow[:, :])
            nc.gpsimd.partition_broadcast(bv_bc[:], bv_row[0:1, :])
            for b in range(B):
                vv = v65[b][:].rearrange("p (t u c) -> p t u c", u=2, c=65)
                nc.sync.dma_start(
                    out=vv[:, :, :, 64],
                    in_=onesb.ap().rearrange("p (t u) -> p t u", u=2))

            with (
                tc.tile_pool(name="xp", bufs=3) as xp,
                tc.tile_pool(name="att", bufs=6) as ap_,
                tc.tile_pool(name="stage", bufs=4) as stp,
                tc.tile_pool(name="small", bufs=4) as smp,
                tc.tile_pool(name="aosb", bufs=2) as aop,
                tc.tile_pool(name="recv", bufs=1) as rcp,
                tc.tile_pool(name="mmps", bufs=2, space="PSUM") as mmp,
                tc.tile_pool(name="scps", bufs=2, space="PSUM") as scp_,
                tc.tile_pool(name="otps", bufs=1, space="PSUM") as otp_,
                # PSUM: scores 2x2 banks + otp 1x2 banks + mm 2 = 8
            ):
                def proj_chunk(b, n):
                    """qkv projection 512-row chunk n of batch b.

                    k/q: weight-stationary N=512 chains -> [feat, rows].
                    v: x-tile stationary so the PSUM comes out
                    [128 rows, 128 feat] and lands in v65 with one
                    tensor_tensor (+bias) per 512-row chunk — no PE
                    transposes. 4 row-chunks of 128 share one PSUM bank.
                    """
                    if True:
                        xsb = xp.tile([128, KT * 512], bf16, tag="xsb",
                                      name=f"xsb{b}_{n}")
                        nc.sync.dma_start(
                            out=xsb[:].rearrange("p (k r) -> p k r", r=512),
                            in_=xT[:, b * S + n * 512:b * S + (n + 1) * 512
                                   ].rearrange("(k p) r -> p k r", p=128))
                        for m in (0, 2):         # k, q feature blocks
                            ps = mmp.tile([128, 512], f32, tag="mm",
                                          name=f"ps{b}_{n}_{m}")
                            for k in range(KT):
                                nc.tensor.matmul(
                                    out=ps[:],
                                    lhsT=wq_sb[:, k * 384 + m * 128:
                                               k * 384 + (m + 1) * 128],
                                    rhs=xsb[:, k * 512:(k + 1) * 512],
                                    start=(k == 0), stop=(k == KT - 1))
                            dst = kT[b] if m == 0 else qT[b]
                            nc.vector.tensor_scalar_add(
                                out=dst[:, n * 512:(n + 1) * 512],
                                in0=ps[:], scalar1=bq_sb[:, m:m + 1])
                        # v: x-tile stationary, direct [rows, feat] layout
                        psv = mmp.tile([128, 512], f32, tag="mm",
                                       name=f"psv{b}_{n}")
                        for rc in range(4):
                            for k in range(KT):
                                # one start clears the whole bank's
                                # has_written bits; later chunks' first
                                # writes overwrite (bits clear), then
                                # accumulate — exact under both bit-clear
                                # and zero-fill semantics
                                nc.tensor.matmul(
                                    out=psv[:, rc * 128:(rc + 1) * 128],
                                    lhsT=xsb[:, k * 512 + rc * 128:
                                             k * 512 + (rc + 1) * 128],
                                    rhs=wq_sb[:, k * 384 + 128:
                                              k * 384 + 256],
                                    start=(rc == 0 and k == 0),
                                    stop=(rc == 3 and k == KT - 1))
                        dstv = v65[b][:, n * 4 * 130:(n + 1) * 4 * 130
                                      ].rearrange("p (t u c) -> p t u c",
                                                  u=2, c=65)
                        nc.vector.tensor_add(
                            dstv[:, :, :, 0:64],
                            psv[:].rearrange("p (t u c) -> p t u c",
                                             u=2, c=64),
                            bv_bc[:].rearrange("p (t u c) -> p t u c",
                                               u=2, c=64))

                def att_pair(b, qc, stg):
                    """Both heads of (batch, q-chunk). The two heads' QK
                    matmuls sit adjacent in the PE queue and run
                    concurrently on row-tiles, writing the two banks of
                    one [128,1024] PSUM tile so a SINGLE FD=1024 exp
                    (with the 1/8 softmax scale folded into its free
                    affine) covers both heads. The PV accumulator is one
                    2-bank [65,1024] tile evacuated to SBUF with one DVE
                    copy right after the last PV; normalization runs
                    from SBUF."""
                    otp = otp_.tile([65, 1024], f32, tag="ot",
                                    name=f"ot{b}_{qc}")
                    ats = {}

                    def emit_qk_exp(kt):
                        sc = scp_.tile([128, 1024], f32, tag="sc",
                                       name=f"sc{b}_{qc}_{kt}")
                        # "qk1": timing-only ablation, emit just head 0
                        for h in range(1 if "qk1" in ablate else 2):
                            nc.tensor.matmul(
                                out=sc[:, h * 512:(h + 1) * 512],
                                lhsT=kT[b][h * 64:(h + 1) * 64,
                                           kt * 128:(kt + 1) * 128],
                                rhs=qT[b][h * 64:(h + 1) * 64,
                                          qc * 512:(qc + 1) * 512],
                                start=True, stop=True)
                        at = ap_.tile([128, 1024], bf16, tag="at",
                                      name=f"at{b}_{qc}_{kt}")
                        # q,k are unscaled; 1/sqrt(Hd) rides the free affine
                        nc.scalar.activation(out=at[:], in_=sc[:],
                                             func=AF.Exp, scale=0.125)
                        ats[kt] = at

                    def emit_pv(kt, h):
                        nc.tensor.matmul(
                            out=otp[:, h * 512:(h + 1) * 512],
                            lhsT=v65[b][:, kt * 130 + h * 65:
                                        kt * 130 + (h + 1) * 65],
                            rhs=ats[kt][:, h * 512:(h + 1) * 512],
                            start=(kt == 0), stop=(kt == SK - 1))

                    nh = 1 if "pv1" in ablate else 2
                    emit_qk_exp(0)
                    for kt in range(1, SK):
                        emit_qk_exp(kt)
                        for h in range(nh):
                            emit_pv(kt - 1, h)
                    for h in range(nh):
                        emit_pv(SK - 1, h)
                    ao = aop.tile([65, 1024], f32, tag="ao",
                                  name=f"ao{b}_{qc}")
                    nc.vector.tensor_copy(ao[:], otp[:])
                    for h in range(2):
                        rc = smp.tile([1, 512], f32, tag="rc",
                                      name=f"rc{b}_{h}_{qc}")
                        nc.vector.reciprocal(
                            rc[:], ao[64:65, h * 512:(h + 1) * 512])
                        bcs = smp.tile([64, 512], f32, tag="bcs",
                                       name=f"bcs{b}_{h}_{qc}")
                        nc.gpsimd.partition_broadcast(bcs[:], rc[0:1, :])
                        nc.vector.tensor_mul(
                            stg[h * 64:(h + 1) * 64, :],
                            ao[0:64, h * 512:(h + 1) * 512], bcs[:])

                def att_all(it, ain, prev, nxt):
                    """All q chunks; shard j = 2b + qc//2, cols (qc%2)*512.

                    Projection chunks for the NEXT batch (and, at b=3, for
                    the next iteration) and the previous iteration's
                    out-projection chains are interleaved one-per-q-chunk
                    so the in-order PE queue always has matmul work while
                    the ACT engine chews on the exps, and the ACT engine
                    never waits at a batch boundary for a big projection
                    block to drain."""
                    rcv_t = None
                    for b in range(B):
                        if b == 1 and prev is not None:
                            # a2a(it-1) has had all of b=0 to complete
                            rcv_t = rcp.tile([128, 8 * 1024], bf16,
                                             tag="rcv", name=f"rcv{it}")
                            for jj in range(8):
                                recv_shard(prev, rcv_t, jj)
                        for qc in range(4):
                            stg = stp.tile([128, 512], bf16, tag="stg",
                                           name=f"stg{it}_{b}_{qc}")
                            att_pair(b, qc, stg)
                            j = 2 * b + qc // 2
                            c0 = (qc % 2) * 512
                            nc.sync.dma_start(
                                out=ain[j * 128:(j + 1) * 128, c0:c0 + 512],
                                in_=stg[:])
                            if prev is not None and b in (1, 2):
                                ci = (b - 1) * 4 + qc
                                out_proj_pair(rcv_t, it - 1, ci)
                            if b + 1 < B:
                                proj_chunk(b + 1, qc)
                            elif nxt:
                                proj_chunk(0, qc)

                def a2a(ain, aout):
                    if single or "nocoll" in ablate:
                        nc.sync.dma_start(out=aout[:], in_=ain[:])
                    else:
                        nc.gpsimd.collective_compute(
                            "AllToAll", mybir.AluOpType.bypass,
                            replica_groups=[list(range(NC))],
                            ins=[ain.opt()], outs=[aout.opt()])

                def recv_shard(aout, rcv, j):
                    nc.sync.dma_start(
                        out=rcv[:, j * 1024:(j + 1) * 1024],
                        in_=aout[j * 128:(j + 1) * 128, :])

                def out_proj(aout, it):
                    """Out-projection for all my 1024 rows from aout."""
                    recv = rcp.tile([128, 8 * 1024], bf16, tag="rcv",
                                    name=f"rcv{it}")
                    for j in range(8):
                        recv_shard(aout, recv, j)
                    out_proj_mms(recv, it)

                def out_proj_pair(ao, it, ci):
                    for t in (2 * ci, 2 * ci + 1):
                        mm, n2 = t // 2, t % 2
                        op = mmp.tile([128, 512], f32, tag="mm",
                                      name=f"op{it}_{mm}_{n2}")
                        for j in range(KT):
                            nc.tensor.matmul(
                                out=op[:],
                                lhsT=ao[:, j * 1024 + mm * 128:
                                        j * 1024 + (mm + 1) * 128],
                                rhs=wout_sb[:, j * 1024 + n2 * 512:
                                            j * 1024 + (n2 + 1) * 512],
                                start=(j == 0), stop=(j == KT - 1))
                        ob = smp.tile([128, 512], f32, tag="ob",
                                      name=f"ob{it}_{mm}_{n2}")
                        nc.vector.tensor_add(
                            ob[:], op[:],
                            bout_bc[:, n2 * 512:(n2 + 1) * 512])
                        nc.sync.dma_start(
                            out=out[mm * 128:(mm + 1) * 128,
                                    n2 * 512:(n2 + 1) * 512],
                            in_=ob[:])

                def out_proj_mms(ao, it):
                    for ci in range(8):
                        out_proj_pair(ao, it, ci)

                prev_aout = None
                for it in range(iters):
                    ain = dp.tile([NC * 128, 1024], bf16, name=f"ain{it}")
                    aout = dp.tile([NC * 128, 1024], bf16, name=f"aout{it}")
                    if it == 0:
                        nc.sync.dma_start(
                            out=wout_sb[:].rearrange("p (k n) -> p k n",
                                                     n=1024),
                            in_=wout.ap().rearrange("(k p) n -> p k n",
                                                    p=128))
                        nc.sync.dma_start(out=bout_sb[:], in_=bout[:, :])
                        nc.gpsimd.partition_broadcast(bout_bc[:],
                                                      bout_sb[0:1, :])
                        for n in range(4):
                            proj_chunk(0, n)
                    att_all(it, ain, prev_aout, nxt=(it + 1 < iters))
                    a2a(ain, aout)
                    prev_aout = aout
                out_proj(prev_aout, iters - 1)

    nc.compile()
    return nc


def _get_nc(iters=1, ablate=(), single=False):
    key = f"nc{iters}{sorted(ablate)}{single}"
    if key not in _CACHE:
        _CACHE[key] = _build(iters, ablate, single)
    return _CACHE[key]


def _make_in_maps(x, w_qkv, b_qkv, w_out, b_out):
    import ml_dtypes
    onesb = np.ones((128, 32), dtype=ml_dtypes.bfloat16)
    x = np.asarray(x, dtype=np.float32)
    w_qkv = np.asarray(w_qkv, dtype=np.float32)
    b_qkv = np.asarray(b_qkv, dtype=np.float32)
    xT = np.ascontiguousarray(
        x.reshape(ROWS, D).T).astype(ml_dtypes.bfloat16)
    bout = np.ascontiguousarray(
        np.asarray(b_out, dtype=np.float32)[None, :])
    woutc = np.ascontiguousarray(
        np.asarray(w_out, dtype=np.float32)).astype(ml_dtypes.bfloat16)
    in_maps = []
    for c in range(NC):
        f0 = 2 * c * 64                      # first feature of head 2c
        wk = w_qkv[:, D + f0:D + f0 + 128]
        wv = w_qkv[:, 2 * D + f0:2 * D + f0 + 128]
        wq = w_qkv[:, f0:f0 + 128]           # unscaled; 1/8 applied at exp
        wc = np.ascontiguousarray(
            np.concatenate([wk, wv, wq], axis=1)).astype(ml_dtypes.bfloat16)
        bk = b_qkv[D + f0:D + f0 + 128]
        bv = b_qkv[2 * D + f0:2 * D + f0 + 128]
        bq = b_qkv[f0:f0 + 128]
        bc = np.ascontiguousarray(
            np.concatenate([bk, bv, bq])[:, None]).astype(np.float32)
        bvrow = np.ascontiguousarray(np.tile(bv, 4)[None, :]).astype(
            np.float32)
        in_maps.append({
            "xT": xT, "wqkv": wc, "bqkv": bc, "bvrow": bvrow,
            "wout": woutc, "bout": bout, "onesb": onesb,
        })
    return in_maps


def kernel(x, w_qkv, b_qkv, w_out, b_out):
    from concourse import bass_utils

    x = np.asarray(x, dtype=np.float32)
    in_maps = _make_in_maps(x, np.asarray(w_qkv), np.asarray(b_qkv),
                            np.asarray(w_out), np.asarray(b_out))
    nc = _get_nc()
    res = bass_utils.run_bass_kernel_spmd(nc, in_maps,
                                          core_ids=list(range(NC)))
    full = np.empty((B, S, D), dtype=np.float32)
    for c in range(NC):
        o = res.results[c]["out"]            # [1024, 1024] rows block c
        b, sh = c // 2, c % 2
        full[b, sh * 1024:(sh + 1) * 1024, :] = o
    return full


# revision 23
# speedup vs baseline: 1.1711x; 1.1711x over previous
"""Multi-head attention forward on 8 TRN2 NeuronCores — tensor-parallel.

Problem: B=4, S=2048, D=1024, H=16, Hd=64, fp32.
  qkv = x @ w_qkv + b_qkv ; per-head softmax(q k^T / 8) v ; out proj.

Sharding: head-parallel. Core c owns heads 2c, 2c+1 and receives the
FULL x (replicated input); it computes q,k,v for its two heads over all
B*S = 8192 rows locally, so attention needs NO collective at all. The
only communication is a small AllToAll of the normalized per-head
attention outputs (bf16, 2 x 1 MB per core) that reshards from
head-blocks to row-blocks, after which the out-projection is fully
local over the core's 1024 output rows (batch c//2, s-half c%2).
There is ONE AllToAll per iteration; its dependent out-projection is
emitted one iteration late so the PE queue never waits on the
collective, and iterations pipeline.

Precision: everything bf16 with fp32 PSUM accumulation. NOTE: random
per-element perturbations of the attention weights do NOT wash out in
the output (signal and error both scale as sqrt(sum a^2) for random v),
so fp8 anywhere in the q/k/score path costs ~3-5% rel err — measured,
not theoretical. bf16 (~0.3%) is the floor that fits the 2e-2 gate.

Layouts (feature-on-partition; h0 on partitions 0:64, h1 on 64:128):
  xT       [1024, 8192]   full x, transposed on host (bf16)
  qT/kT[b] [128, 2048]    per-batch projections; UNSCALED q (the
                          1/sqrt(Hd) rides the exp free affine)
  v65[b]   [128, 16*130]  per k-tile: [v_h0(64)|1|v_h1(64)|1] so PSUM
                          row 64 of each head's PV yields the softmax
                          denominator for free. Written directly from
                          the v-proj PSUM ([128 rows, 128 feat],
                          x-tile stationary) — no PE transposes.
  sc       [128 k, 1024]  fp32 PSUM (2 banks): h0 cols 0:512, h1
                          512:1024; ONE FD=1024 exp per k-tile covers
                          both heads (halves ACT instruction count).
  otp      [65, 1024]     fp32 PSUM (2 banks): PV accumulator for both
                          heads; row 64 = denominator. Evacuated to
                          SBUF by one DVE copy so the banks free fast;
                          normalization runs from SBUF.
  staging  [128, 512]     normalized attn-out, both heads -> A2A shard
                          (shard j = 2b + qc//2, columns (qc%2)*512)

The two heads' QK matmuls are emitted adjacently so they run
concurrently as 64-row PE tiles (h0 partitions 0:64 -> tile T0, h1
64:128 -> T8). PSUM budget: scores 2x2 banks + otp 1x2 + proj/out-proj
mm 2 = 8. Out-projection is all-bf16 (attn-out used straight from the
A2A receive, no fp32 upconvert); its bias is a broadcast DVE add fused
into the PSUM evacuation, not a matmul.
"""

import sys

import numpy as np

for _p in ("/opt/trn_rl_repo",):
    if _p not in sys.path:
        sys.path.insert(0, _p)

B, S, D = 4, 2048, 1024
H, HD = 16, 64
NC = 8
ROWS = B * S           # 8192 total rows
MYROWS = ROWS // NC    # 1024 output rows per core
KT = D // 128          # 8 k-tiles over D

_CACHE = {}


def _build(iters=1, ablate=(), single=False):
    import concourse.bass as bass  # noqa: F401
    import concourse.mybir as mybir
    from concourse import bacc, tile

    dt = mybir.dt
    f32, bf16 = dt.float32, dt.bfloat16
    AF = mybir.ActivationFunctionType

    nc = bacc.Bacc("TRN2", target_bir_lowering=False, debug=False,
                   num_devices=(1 if single else NC))

    xT = nc.dram_tensor("xT", [D, ROWS], bf16, kind="ExternalInput")
    wqkv = nc.dram_tensor("wqkv", [D, 384], bf16, kind="ExternalInput")
    bqkv = nc.dram_tensor("bqkv", [384, 1], f32, kind="ExternalInput")
    bvrow = nc.dram_tensor("bvrow", [1, 512], f32, kind="ExternalInput")
    wout = nc.dram_tensor("wout", [D, D], bf16, kind="ExternalInput")
    bout = nc.dram_tensor("bout", [1, D], f32, kind="ExternalInput")
    onesb = nc.dram_tensor("onesb", [128, 32], bf16, kind="ExternalInput")
    out = nc.dram_tensor("out", [MYROWS, D], f32, kind="ExternalOutput")

    SK = 16                 # key tiles of 128 per batch
    with tile.TileContext(nc) as tc:
        with (
            tc.tile_pool(name="persist", bufs=1) as pp,
            tc.tile_pool(name="dram", bufs=1, space="DRAM") as dp,
        ):
            wq_sb = pp.tile([128, KT * 384], bf16)
            bq_sb = pp.tile([128, 3], f32)
            bv_row = pp.tile([1, 512], f32)
            bv_bc = pp.tile([128, 512], f32)
            wout_sb = pp.tile([128, KT * 1024], bf16)
            bout_sb = pp.tile([1, 1024], f32)
            bout_bc = pp.tile([128, 1024], f32)
            qT = [pp.tile([128, S], bf16, name=f"qT{b}") for b in range(B)]
            kT = [pp.tile([128, S], bf16, name=f"kT{b}") for b in range(B)]
            v65 = [pp.tile([128, SK * 130], bf16, name=f"v65_{b}")
                   for b in range(B)]

            nc.sync.dma_start(
                out=wq_sb[:].rearrange("p (k m) -> p k m", m=384),
                in_=wqkv.ap().rearrange("(k p) m -> p k m", p=128))
            nc.sync.dma_start(
                out=bq_sb[:],
                in_=bqkv.ap().rearrange("(m p) o -> p (m o)", p=128))
            nc.sync.dma_start(out=bv_row[:], in_=bvrow[:, :])
            nc.gpsimd.partition_broadcast(bv_bc[:], bv_row[0:1, :])
            for b in range(B):
                vv = v65[b][:].rearrange("p (t u c) -> p t u c", u=2, c=65)
                nc.sync.dma_start(
                    out=vv[:, :, :, 64],
                    in_=onesb.ap().rearrange("p (t u) -> p t u", u=2))

            with (
                tc.tile_pool(name="xp", bufs=3) as xp,
                tc.tile_pool(name="att", bufs=6) as ap_,
                tc.tile_pool(name="stage", bufs=4) as stp,
                tc.tile_pool(name="small", bufs=4) as smp,
                tc.tile_pool(name="aosb", bufs=2) as aop,
                tc.tile_pool(name="recv", bufs=1) as rcp,
                tc.tile_pool(name="mmps", bufs=2, space="PSUM") as mmp,
                tc.tile_pool(name="scps", bufs=2, space="PSUM") as scp_,
                tc.tile_pool(name="otps", bufs=1, space="PSUM") as otp_,
                # PSUM: scores 2x2 banks + otp 1x2 banks + mm 2 = 8
            ):
                fillers = []

                def run_fill(k=1):
                    for _ in range(k):
                        if fillers:
                            fillers.pop(0)()

                def proj_kq_chain(b, n, m, xsb):
                    ps = mmp.tile([128, 512], f32, tag="mm",
                                  name=f"ps{b}_{n}_{m}")
                    for k in range(KT):
                        nc.tensor.matmul(
                            out=ps[:],
                            lhsT=wq_sb[:, k * 384 + m * 128:
                                       k * 384 + (m + 1) * 128],
                            rhs=xsb[:, k * 512:(k + 1) * 512],
                            start=(k == 0), stop=(k == KT - 1))
                    dst = kT[b] if m == 0 else qT[b]
                    nc.vector.tensor_scalar_add(
                        out=dst[:, n * 512:(n + 1) * 512],
                        in0=ps[:], scalar1=bq_sb[:, m:m + 1])

                def proj_v_chain(b, n, xsb):
                    # v: x-tile stationary, direct [rows, feat] layout.
                    # 4 row-chunks of 128 share one PSUM bank (one start
                    # clears the whole bank's has_written bits; later
                    # chunks' first writes overwrite, then accumulate —
                    # exact under both bit-clear and zero-fill semantics)
                    psv = mmp.tile([128, 512], f32, tag="mm",
                                   name=f"psv{b}_{n}")
                    for rc in range(4):
                        for k in range(KT):
                            nc.tensor.matmul(
                                out=psv[:, rc * 128:(rc + 1) * 128],
                                lhsT=xsb[:, k * 512 + rc * 128:
                                         k * 512 + (rc + 1) * 128],
                                rhs=wq_sb[:, k * 384 + 128:
                                          k * 384 + 256],
                                start=(rc == 0 and k == 0),
                                stop=(rc == 3 and k == KT - 1))
                    dstv = v65[b][:, n * 4 * 130:(n + 1) * 4 * 130
                                  ].rearrange("p (t u c) -> p t u c",
                                              u=2, c=65)
                    nc.vector.tensor_add(
                        dstv[:, :, :, 0:64],
                        psv[:].rearrange("p (t u c) -> p t u c",
                                         u=2, c=64),
                        bv_bc[:].rearrange("p (t u c) -> p t u c",
                                           u=2, c=64))

                def push_proj_chunk(b, n):
                    """Queue projection chunk n of batch b as three PE
                    filler chains (k, q, v); the x DMA is issued now so
                    it is in flight well before the chains run."""
                    xsb = xp.tile([128, KT * 512], bf16, tag="xsb",
                                  name=f"xsb{b}_{n}")
                    nc.sync.dma_start(
                        out=xsb[:].rearrange("p (k r) -> p k r", r=512),
                        in_=xT[:, b * S + n * 512:b * S + (n + 1) * 512
                               ].rearrange("(k p) r -> p k r", p=128))
                    fillers.append(lambda: proj_kq_chain(b, n, 0, xsb))
                    fillers.append(lambda: proj_kq_chain(b, n, 2, xsb))
                    fillers.append(lambda: proj_v_chain(b, n, xsb))

                def proj_chunk(b, n):
                    push_proj_chunk(b, n)
                    run_fill(3)

                def att_pair(b, qc, stg):
                    """Both heads of (batch, q-chunk). The two heads' QK
                    matmuls sit adjacent in the PE queue and run
                    concurrently on row-tiles, writing the two banks of
                    one [128,1024] PSUM tile so a SINGLE FD=1024 exp
                    (with the 1/8 softmax scale folded into its free
                    affine) covers both heads. The PV accumulator is one
                    2-bank [65,1024] tile evacuated to SBUF with one DVE
                    copy right after the last PV; normalization runs
                    from SBUF."""
                    otp = otp_.tile([65, 1024], f32, tag="ot",
                                    name=f"ot{b}_{qc}")
                    ats = {}

                    def emit_qk_exp(kt):
                        sc = scp_.tile([128, 1024], f32, tag="sc",
                                       name=f"sc{b}_{qc}_{kt}")
                        # "qk1": timing-only ablation, emit just head 0
                        for h in range(1 if "qk1" in ablate else 2):
                            nc.tensor.matmul(
                                out=sc[:, h * 512:(h + 1) * 512],
                                lhsT=kT[b][h * 64:(h + 1) * 64,
                                           kt * 128:(kt + 1) * 128],
                                rhs=qT[b][h * 64:(h + 1) * 64,
                                          qc * 512:(qc + 1) * 512],
                                start=True, stop=True)
                        at = ap_.tile([128, 1024], bf16, tag="at",
                                      name=f"at{b}_{qc}_{kt}")
                        # q,k are unscaled; 1/sqrt(Hd) rides the free affine
                        nc.scalar.activation(out=at[:], in_=sc[:],
                                             func=AF.Exp, scale=0.125)
                        ats[kt] = at

                    def emit_pv(kt, h):
                        nc.tensor.matmul(
                            out=otp[:, h * 512:(h + 1) * 512],
                            lhsT=v65[b][:, kt * 130 + h * 65:
                                        kt * 130 + (h + 1) * 65],
                            rhs=ats[kt][:, h * 512:(h + 1) * 512],
                            start=(kt == 0), stop=(kt == SK - 1))

                    nh = 1 if "pv1" in ablate else 2
                    emit_qk_exp(0)
                    for kt in range(1, SK):
                        emit_qk_exp(kt)
                        for h in range(nh):
                            emit_pv(kt - 1, h)
                        if kt % 4 == 0:
                            # thread one queued proj/out-proj chain into
                            # the PE stream while ACT chews on the exps
                            run_fill(1)
                    for h in range(nh):
                        emit_pv(SK - 1, h)
                    run_fill(1)
                    ao = aop.tile([65, 1024], f32, tag="ao",
                                  name=f"ao{b}_{qc}")
                    nc.vector.tensor_copy(ao[:], otp[:])
                    for h in range(2):
                        rc = smp.tile([1, 512], f32, tag="rc",
                                      name=f"rc{b}_{h}_{qc}")
                        nc.vector.reciprocal(
                            rc[:], ao[64:65, h * 512:(h + 1) * 512])
                        bcs = smp.tile([64, 512], f32, tag="bcs",
                                       name=f"bcs{b}_{h}_{qc}")
                        nc.gpsimd.partition_broadcast(bcs[:], rc[0:1, :])
                        nc.vector.tensor_mul(
                            stg[h * 64:(h + 1) * 64, :],
                            ao[0:64, h * 512:(h + 1) * 512], bcs[:])

                def att_all(it, ain, prev, nxt):
                    """All q chunks; shard j = 2b + qc//2, cols (qc%2)*512.

                    Projection chunks for the NEXT batch (and, at b=3, for
                    the next iteration) and the previous iteration's
                    out-projection chains are interleaved one-per-q-chunk
                    so the in-order PE queue always has matmul work while
                    the ACT engine chews on the exps, and the ACT engine
                    never waits at a batch boundary for a big projection
                    block to drain."""
                    rcv_t = None
                    for b in range(B):
                        if b == 1 and prev is not None:
                            # a2a(it-1) has had all of b=0 to complete
                            rcv_t = rcp.tile([128, 8 * 1024], bf16,
                                             tag="rcv", name=f"rcv{it}")
                            for jj in range(8):
                                recv_shard(prev, rcv_t, jj)
                        for qc in range(4):
                            if b + 1 < B:
                                push_proj_chunk(b + 1, qc)
                            elif nxt:
                                push_proj_chunk(0, qc)
                            if prev is not None and b in (1, 2):
                                ci = (b - 1) * 4 + qc
                                fillers.append(
                                    lambda c=ci: out_proj_pair(
                                        rcv_t, it - 1, c))
                            stg = stp.tile([128, 512], bf16, tag="stg",
                                           name=f"stg{it}_{b}_{qc}")
                            att_pair(b, qc, stg)
                            j = 2 * b + qc // 2
                            c0 = (qc % 2) * 512
                            nc.sync.dma_start(
                                out=ain[j * 128:(j + 1) * 128, c0:c0 + 512],
                                in_=stg[:])
                        run_fill(2)
                    run_fill(len(fillers))

                def a2a(ain, aout):
                    if single or "nocoll" in ablate:
                        nc.sync.dma_start(out=aout[:], in_=ain[:])
                    else:
                        nc.gpsimd.collective_compute(
                            "AllToAll", mybir.AluOpType.bypass,
                            replica_groups=[list(range(NC))],
                            ins=[ain.opt()], outs=[aout.opt()])

                def recv_shard(aout, rcv, j):
                    nc.sync.dma_start(
                        out=rcv[:, j * 1024:(j + 1) * 1024],
                        in_=aout[j * 128:(j + 1) * 128, :])

                def out_proj(aout, it):
                    """Out-projection for all my 1024 rows from aout."""
                    recv = rcp.tile([128, 8 * 1024], bf16, tag="rcv",
                                    name=f"rcv{it}")
                    for j in range(8):
                        recv_shard(aout, recv, j)
                    out_proj_mms(recv, it)

                def out_proj_pair(ao, it, ci):
                    for t in (2 * ci, 2 * ci + 1):
                        mm, n2 = t // 2, t % 2
                        op = mmp.tile([128, 512], f32, tag="mm",
                                      name=f"op{it}_{mm}_{n2}")
                        for j in range(KT):
                            nc.tensor.matmul(
                                out=op[:],
                                lhsT=ao[:, j * 1024 + mm * 128:
                                        j * 1024 + (mm + 1) * 128],
                                rhs=wout_sb[:, j * 1024 + n2 * 512:
                                            j * 1024 + (n2 + 1) * 512],
                                start=(j == 0), stop=(j == KT - 1))
                        ob = smp.tile([128, 512], f32, tag="ob",
                                      name=f"ob{it}_{mm}_{n2}")
                        nc.vector.tensor_add(
                            ob[:], op[:],
                            bout_bc[:, n2 * 512:(n2 + 1) * 512])
                        nc.sync.dma_start(
                            out=out[mm * 128:(mm + 1) * 128,
                                    n2 * 512:(n2 + 1) * 512],
                            in_=ob[:])

                def out_proj_mms(ao, it):
                    for ci in range(8):
                        out_proj_pair(ao, it, ci)

                prev_aout = None
                for it in range(iters):
                    ain = dp.tile([NC * 128, 1024], bf16, name=f"ain{it}")
                    aout = dp.tile([NC * 128, 1024], bf16, name=f"aout{it}")
                    if it == 0:
                        nc.sync.dma_start(
                            out=wout_sb[:].rearrange("p (k n) -> p k n",
                                                     n=1024),
                            in_=wout.ap().rearrange("(k p) n -> p k n",
                                                    p=128))
                        nc.sync.dma_start(out=bout_sb[:], in_=bout[:, :])
                        nc.gpsimd.partition_broadcast(bout_bc[:],
                                                      bout_sb[0:1, :])
                        for n in range(4):
                            proj_chunk(0, n)
                    att_all(it, ain, prev_aout, nxt=(it + 1 < iters))
                    a2a(ain, aout)
                    prev_aout = aout
                out_proj(prev_aout, iters - 1)

    nc.compile()
    return nc


def _get_nc(iters=1, ablate=(), single=False):
    key = f"nc{iters}{sorted(ablate)}{single}"
    if key not in _CACHE:
        _CACHE[key] = _build(iters, ablate, single)
    return _CACHE[key]


def _make_in_maps(x, w_qkv, b_qkv, w_out, b_out):
    import ml_dtypes
    onesb = np.ones((128, 32), dtype=ml_dtypes.bfloat16)
    x = np.asarray(x, dtype=np.float32)
    w_qkv = np.asarray(w_qkv, dtype=np.float32)
    b_qkv = np.asarray(b_qkv, dtype=np.float32)
    xT = np.ascontiguousarray(
        x.reshape(ROWS, D).T).astype(ml_dtypes.bfloat16)
    bout = np.ascontiguousarray(
        np.asarray(b_out, dtype=np.float32)[None, :])
    woutc = np.ascontiguousarray(
        np.asarray(w_out, dtype=np.float32)).astype(ml_dtypes.bfloat16)
    in_maps = []
    for c in range(NC):
        f0 = 2 * c * 64                      # first feature of head 2c
        wk = w_qkv[:, D + f0:D + f0 + 128]
        wv = w_qkv[:, 2 * D + f0:2 * D + f0 + 128]
        wq = w_qkv[:, f0:f0 + 128]           # unscaled; 1/8 applied at exp
        wc = np.ascontiguousarray(
            np.concatenate([wk, wv, wq], axis=1)).astype(ml_dtypes.bfloat16)
        bk = b_qkv[D + f0:D + f0 + 128]
        bv = b_qkv[2 * D + f0:2 * D + f0 + 128]
        bq = b_qkv[f0:f0 + 128]
        bc = np.ascontiguousarray(
            np.concatenate([bk, bv, bq])[:, None]).astype(np.float32)
        bvrow = np.ascontiguousarray(np.tile(bv, 4)[None, :]).astype(
            np.float32)
        in_maps.append({
            "xT": xT, "wqkv": wc, "bqkv": bc, "bvrow": bvrow,
            "wout": woutc, "bout": bout, "onesb": onesb,
        })
    return in_maps


def kernel(x, w_qkv, b_qkv, w_out, b_out):
    from concourse import bass_utils

    x = np.asarray(x, dtype=np.float32)
    in_maps = _make_in_maps(x, np.asarray(w_qkv), np.asarray(b_qkv),
                            np.asarray(w_out), np.asarray(b_out))
    nc = _get_nc()
    res = bass_utils.run_bass_kernel_spmd(nc, in_maps,
                                          core_ids=list(range(NC)))
    full = np.empty((B, S, D), dtype=np.float32)
    for c in range(NC):
        o = res.results[c]["out"]            # [1024, 1024] rows block c
        b, sh = c // 2, c % 2
        full[b, sh * 1024:(sh + 1) * 1024, :] = o
    return full
